# revision 3
# baseline (speedup 1.0000x reference)
"""Trainium2 Bass kernel v4 for nn_AreaLoss_7069516169625 (topk_masking).

loss = sum(p)/denom + sum_b sum_{c in ranks 3..24 of main_out[b]} sum(features[b,c]) / denom

v4 = v3 (bf16 features/p, balanced reduce split, p load on gpsimd) with the
flat top-25 value ranking replaced by a two-level hierarchy:

- Phase A: main_out is ALSO loaded as [32,125] (8 blocks of 125 per row, one
  block per partition).  4 rounds of max8 + 3 match_replace give each block's
  top-32 values ca[32,32] in ~1/8 the per-pass time of the flat layout (DVE
  passes cost ~1.19ns/free-elem regardless of partition count).
- Chunked rearrange: after A-round r, sync DMAs ca[:,8r:8r+8] ([32,8]) into a
  per-row layout chunk wb*[4, 64r:64r+64].  Row rank 8k-1 has per-block rank
  <= 8k-1, so B-round k only needs chunks 1..k -- every chunk DMA hides
  behind the remaining A rounds.
- Phase B: 4 rounds of max8 over the growing prefix [4,64k] with
  out-of-place match_replace ping-ponging between wbA/wbB (chunk k lands in
  the buffer that round k reads: A,B,A,B).
- Index recovery + gather + reduce: same staged find_index8 windows, flatten
  DMAs, indirect gathers, and DVE+scalar+PE reduce as v1/v3, just ~3us
  earlier.

DVE op order (s_dve counts):
  1 memset; 2 a0; 3 ka1; 4 a1; 5 ka2; 6 a2; 7 ka3; 8 a3;
  9 b1; 10 kb1; 11 b2; 12 mi0; 13 tta(0:8); 14 kb2; 15 b3;
  16 mi1; 17 tta(8:16); 18 mi2; 19 tta(16:21); 20 kb3; 21 b4;
  22 mi3; 23 tta(21:22); 24 reduce
Chunk gates (sync): s_dve>=2,4,6,8 -> c1..c4.  Flatten gates: 13,17,19.
w3 gather gate: 23.
"""

import numpy as np
import ml_dtypes

import concourse.bass as bass
import concourse.mybir as mybir
from concourse.bass_utils import run_bass_kernel_spmd

B, C, H, W = 32, 1000, 56, 56
HW = H * W  # 3136
NCORES = 8
BL = B // NCORES  # 4
NBLK = 8  # blocks per row
BW = C // NBLK  # 125
TOPK, SKIP = 25, 3
SEL = TOPK - SKIP  # 22
NGAT = SEL * BL  # 88
NP = NGAT + BL  # 92
DENOM = float(B * HW)
NEG = -3.0e38
SPLIT = 1546  # bf16 reduce runs 1x on DVE: balance vs scalar
BF16 = mybir.dt.bfloat16

# flattened windows: (idxg col slice, n maps, dst partition)
FWINS = [
    ((0, 8), 32, 0),
    ((8, 16), 32, 32),
    ((16, 21), 20, 64),
]
MARK_FL = [13, 17, 19]
MARK_G3 = 23
MARK_CH = [2, 4, 6, 8]


def build_nc(guard=True) -> bass.Bass:
    nc = bass.Bass(detect_race_conditions=guard)

    feat = nc.declare_dram_parameter("features", [BL * C, HW], BF16, isOutput=False)
    mo = nc.declare_dram_parameter("main_out", [BL, C], mybir.dt.float32, isOutput=False)
    p_in = nc.declare_dram_parameter("p", [BL, HW], BF16, isOutput=False)
    out_ext = nc.declare_dram_parameter("out", [1, 1], mybir.dt.float32, isOutput=True)

    from contextlib import ExitStack

    with ExitStack() as ctx:
        e = ctx.enter_context
        m0r = e(nc.sbuf_tensor([BL * NBLK, BW], mybir.dt.float32))
        ka1 = e(nc.sbuf_tensor([BL * NBLK, BW], mybir.dt.float32))
        ka2 = e(nc.sbuf_tensor([BL * NBLK, BW], mybir.dt.float32))
        ka3 = e(nc.sbuf_tensor([BL * NBLK, BW], mybir.dt.float32))
        ca = e(nc.sbuf_tensor([BL * NBLK, 32], mybir.dt.float32))
        wba = e(nc.sbuf_tensor([BL, 256], mybir.dt.float32))
        wbb = e(nc.sbuf_tensor([BL, 256], mybir.dt.float32))
        m0 = e(nc.sbuf_tensor([BL, C], mybir.dt.float32))
        vals = e(nc.sbuf_tensor([BL, 32], mybir.dt.float32))
        idx = e(nc.sbuf_tensor([BL, 32], mybir.dt.uint32))
        idxg = e(nc.sbuf_tensor([BL, 32], mybir.dt.uint32))
        idxc0 = e(nc.sbuf_tensor([32, 1], mybir.dt.uint32))
        idxc1 = e(nc.sbuf_tensor([32, 1], mybir.dt.uint32))
        idxc2 = e(nc.sbuf_tensor([20, 1], mybir.dt.uint32))
        rowb = e(nc.sbuf_tensor([BL, 1], mybir.dt.uint32))
        gat = e(nc.sbuf_tensor([NP, HW], BF16))
        dump = e(nc.sbuf_tensor([NP, HW - SPLIT], BF16))
        colsum = e(nc.sbuf_tensor([NP, 2], mybir.dt.float32))
        ones = e(nc.sbuf_tensor([NP, 1], mybir.dt.float32))
        res = e(nc.sbuf_tensor([1, 1], mybir.dt.float32))
        warm = e(nc.sbuf_tensor([1, 1], mybir.dt.float32))
        acc = e(nc.psum_tensor([1, 2], mybir.dt.float32))
        s_mo = e(nc.semaphore())
        s_m0r = e(nc.semaphore())
        s_ck = e(nc.semaphore())
        s_p = e(nc.semaphore())
        s_out = e(nc.semaphore())
        s_gat = e(nc.semaphore())
        s_mm = e(nc.semaphore())
        s_act = e(nc.semaphore())
        s_red = e(nc.semaphore())
        s_rb = e(nc.semaphore())
        s_fl0 = e(nc.semaphore())
        s_fl1 = e(nc.semaphore())
        s_fl2 = e(nc.semaphore())
        s_dve = e(nc.semaphore())
        block = e(nc.Block())

        marks = {}
        idxcs = [idxc0, idxc1, idxc2]
        s_fls = [s_fl0, s_fl1, s_fl2]
        # chunk k (0-based) destination: rounds read A,B,A,B
        chunk_dst = [wba, wbb, wba, wbb]

        @block.sync
        def _(sync):
            sync.dma_start(m0r[:], mo[:]).then_inc(s_m0r, 16)
            sync.dma_start(m0[:], mo[:]).then_inc(s_mo, 16)
            for k in range(4):
                sync.wait_ge(s_dve, MARK_CH[k])
                sync.dma_start(
                    chunk_dst[k][:, 64 * k : 64 * k + 64], ca[:, 8 * k : 8 * k + 8]
                ).then_inc(s_ck, 16)
            for w, ((c0, c1), nmaps, part) in enumerate(FWINS):
                sync.wait_ge(s_dve, MARK_FL[w])
                with nc.allow_non_contiguous_dma(reason="index flatten"):
                    sync.dma_start(idxcs[w][:], idxg[:, c0:c1]).then_inc(s_fls[w], 16)
            sync.wait_ge(s_red, 1)
            sync.dma_start(out_ext[:], res[:]).then_inc(s_out, 16)

        @block.vector
        def _(vector):
            n = 0

            def step(emit):
                nonlocal n
                if guard and n:
                    vector.wait_ge(s_dve, n)
                inst = emit()
                inst.then_inc(s_dve, 1)
                n += 1
                return inst

            def tta(c0, c1):
                step(
                    lambda: vector.tensor_tensor(
                        out=idxg[:, c0:c1],
                        in0=idx[:, c0:c1],
                        in1=rowb[:].to_broadcast([BL, c1 - c0]),
                        op=mybir.AluOpType.add,
                    )
                )

            def mi(lo, hi, d):
                step(
                    lambda: vector.max_index(
                        idx[0:BL, d : d + 8], vals[:, lo:hi], m0[:]
                    )
                )

            step(lambda: vector.memset(ones[:], 1.0 / DENOM))  # 1
            vector.wait_ge(s_m0r, 16)
            vector.wait_ge(s_rb, 1)
            # phase A: per-block top-32
            step(lambda: vector.max(ca[:, 0:8], m0r[:]))  # 2
            step(lambda: vector.match_replace(ka1[:], ca[:, 0:8], m0r[:], NEG))  # 3
            step(lambda: vector.max(ca[:, 8:16], ka1[:]))  # 4
            step(lambda: vector.match_replace(ka2[:], ca[:, 8:16], ka1[:], NEG))  # 5
            step(lambda: vector.max(ca[:, 16:24], ka2[:]))  # 6
            step(lambda: vector.match_replace(ka3[:], ca[:, 16:24], ka2[:], NEG))  # 7
            step(lambda: vector.max(ca[:, 24:32], ka3[:]))  # 8
            # phase B over growing per-row prefix, knock ping-pongs A->B->A->B
            vector.wait_ge(s_ck, 16)
            step(lambda: vector.max(vals[:, 0:8], wba[:, 0:64]))  # 9
            step(
                lambda: vector.match_replace(
                    wbb[:, 0:64], vals[:, 0:8], wba[:, 0:64], NEG
                )
            )  # 10
            vector.wait_ge(s_ck, 32)
            step(lambda: vector.max(vals[:, 8:16], wbb[:, 0:128]))  # 11
            vector.wait_ge(s_mo, 16)
            mi(3, 11, 0)  # 12
            tta(0, 8)  # 13
            step(
                lambda: vector.match_replace(
                    wba[:, 0:128], vals[:, 8:16], wbb[:, 0:128], NEG
                )
            )  # 14
            vector.wait_ge(s_ck, 48)
            step(lambda: vector.max(vals[:, 16:24], wba[:, 0:192]))  # 15
            mi(11, 19, 8)  # 16
            tta(8, 16)  # 17
            mi(16, 24, 13)  # 18
            tta(16, 21)  # 19
            step(
                lambda: vector.match_replace(
                    wbb[:, 0:192], vals[:, 16:24], wba[:, 0:192], NEG
                )
            )  # 20
            vector.wait_ge(s_ck, 64)
            step(lambda: vector.max(vals[:, 24:32], wbb[:, 0:256]))  # 21
            mi(17, 25, 14)  # 22
            tta(21, 22)  # 23
            assert n == MARK_G3, n
            vector.wait_ge(s_gat, 64)
            vector.wait_ge(s_p, 16)
            step(
                lambda: vector.reduce_sum(
                    colsum[:, 0:1], gat[:, 0:SPLIT], axis=mybir.AxisListType.X
                )
            )
            marks["red"] = n
            vector.wait_ge(s_mm, 1)
            if guard:
                vector.wait_ge(s_dve, n)
            vector.reduce_sum(res[:], acc[:], axis=mybir.AxisListType.X).then_inc(
                s_red, 1
            )

        @block.scalar
        def _(scalar):
            scalar.wait_ge(s_dve, 1)
            scalar.activation(
                warm[:], ones[0:1, :], mybir.ActivationFunctionType.Copy
            ).then_inc(s_act, 1)
            scalar.wait_ge(s_gat, 64)
            scalar.wait_ge(s_p, 16)
            scalar.activation(
                dump[:],
                gat[:, SPLIT:HW],
                mybir.ActivationFunctionType.Copy,
                accum_out=colsum[:, 1:2],
            ).then_inc(s_act, 1)

        @block.gpsimd
        def _(gpsimd):
            gpsimd.dma_start(gat[NGAT:NP, :], p_in[:]).then_inc(s_p, 16)
            gpsimd.iota(
                rowb[:], pattern=[[0, 1]], base=0, channel_multiplier=C
            ).then_inc(s_rb, 1)
            for w, ((c0, c1), nmaps, part) in enumerate(FWINS):
                gpsimd.wait_ge(s_fls[w], 16)
                gpsimd.indirect_dma_start(
                    out=gat[part : part + nmaps, :],
                    out_offset=None,
                    in_=feat[:],
                    in_offset=bass.IndirectOffsetOnAxis(ap=idxcs[w][:], axis=0),
                ).then_inc(s_gat, 16)
            # final window: rank 24 of each row -- [4,1] offsets, no flatten
            gpsimd.wait_ge(s_dve, MARK_G3)
            gpsimd.indirect_dma_start(
                out=gat[84:88, :],
                out_offset=None,
                in_=feat[:],
                in_offset=bass.IndirectOffsetOnAxis(ap=idxg[:, 21:22], axis=0),
            ).then_inc(s_gat, 16)

        @block.tensor
        def _(tensor):
            tensor.wait_ge(s_dve, marks["red"])
            tensor.wait_ge(s_act, 2)
            tensor.matmul(acc[:], ones[:], colsum[:]).then_inc(s_mm, 1)

    return nc


def shard_inputs(p, main_out, features):
    p16 = p.astype(ml_dtypes.bfloat16)
    f16 = features.astype(ml_dtypes.bfloat16)
    in_maps = []
    for i in range(NCORES):
        sl = slice(i * BL, (i + 1) * BL)
        in_maps.append(
            {
                "features": f16[sl].reshape(BL * C, HW),
                "main_out": main_out[sl],
                "p": p16[sl].reshape(BL, HW),
            }
        )
    return in_maps


def kernel(p, main_out, features, return_res=False, guard=True):
    p = np.ascontiguousarray(np.asarray(p, dtype=np.float32))
    main_out = np.ascontiguousarray(np.asarray(main_out, dtype=np.float32))
    features = np.ascontiguousarray(np.asarray(features, dtype=np.float32))

    nc = build_nc(guard=guard)
    in_maps = shard_inputs(p, main_out, features)
    res = run_bass_kernel_spmd(nc, in_maps, core_ids=list(range(NCORES)))
    total = np.float32(0.0)
    for r in res.results:
        total += r["out"][0, 0]
    out = np.asarray(total, dtype=np.float32)
    if return_res:
        return out, res
    return out


# revision 5
# speedup vs baseline: 1.0707x; 1.0707x over previous
"""Trainium2 Bass kernel v4 for nn_AreaLoss_7069516169625 (topk_masking).

loss = sum(p)/denom + sum_b sum_{c in ranks 3..24 of main_out[b]} sum(features[b,c]) / denom

Measured on 8 trn2 NeuronCores: ~33.0-33.5us HW exec (neuron-profile),
rel err 1.3e-5 (bf16 data rounding), vs 36.1-42.3us for the v1 baseline.

Structure: bf16 features/p (host cast; halves gather traffic), balanced
DVE/scalar reduce split, p load on gpsimd, and the flat top-25 value
ranking replaced by a two-level hierarchy:

- Phase A: main_out is ALSO loaded as [32,125] (8 blocks of 125 per row, one
  block per partition).  4 rounds of max8 + 3 match_replace give each block's
  top-32 values ca[32,32] in ~1/8 the per-pass time of the flat layout (DVE
  passes cost ~1.19ns/free-elem regardless of partition count).
- Chunked rearrange: after A-round r, sync DMAs ca[:,8r:8r+8] ([32,8]) into a
  per-row layout chunk wb*[4, 64r:64r+64].  Row rank 8k-1 has per-block rank
  <= 8k-1, so B-round k only needs chunks 1..k -- every chunk DMA hides
  behind the remaining A rounds.
- Phase B: 4 rounds of max8 over the growing prefix [4,64k] with
  out-of-place match_replace ping-ponging between wbA/wbB (chunk k lands in
  the buffer that round k reads: A,B,A,B).
- Index recovery + gather + reduce: same staged find_index8 windows, flatten
  DMAs, indirect gathers, and DVE+scalar+PE reduce as v1/v3, just ~3us
  earlier.

v6: mi2/tta2 swapped before mi1/tta1 (both ready after B3) so w2's
flatten->gather hop chain (~3.2us of sem/DMA latency) starts ~1.6us
earlier; sync flattens and gpsimd gathers issue in readiness order
w0, w2, w1, w3.

DVE op order (s_dve counts):
  1 memset; 2 a0; 3 ka1; 4 a1; 5 ka2; 6 a2; 7 ka3; 8 a3;
  9 b1; 10 kb1; 11 b2; 12 mi0; 13 tta(0:8); 14 kb2; 15 b3;
  16 mi1; 17 tta(8:16); 18 mi2; 19 tta(16:21); 20 kb3; 21 b4;
  22 mi3; 23 tta(21:22); 24 reduce
Chunk gates (sync): s_dve>=2,4,6,8 -> c1..c4.  Flatten gates: 13,17,19.
w3 gather gate: 23.
"""

import numpy as np
import ml_dtypes

import concourse.bass as bass
import concourse.mybir as mybir
from concourse.bass_utils import run_bass_kernel_spmd

B, C, H, W = 32, 1000, 56, 56
HW = H * W  # 3136
NCORES = 8
BL = B // NCORES  # 4
NBLK = 8  # blocks per row
BW = C // NBLK  # 125
TOPK, SKIP = 25, 3
SEL = TOPK - SKIP  # 22
NGAT = SEL * BL  # 88
NP = NGAT + BL  # 92
DENOM = float(B * HW)
NEG = -3.0e38
SPLIT = 1546  # bf16 reduce runs 1x on DVE: balance vs scalar
BF16 = mybir.dt.bfloat16

# flattened windows: (idxg col slice, n maps, dst partition)
FWINS = [
    ((0, 8), 32, 0),
    ((8, 16), 32, 32),
    ((16, 21), 20, 64),
]
MARK_FL = [13, 19, 17]
FL_ORDER = [0, 2, 1]
MARK_G3 = 23
MARK_CH = [2, 4, 6, 8]


def build_nc(guard=True) -> bass.Bass:
    nc = bass.Bass(detect_race_conditions=guard)

    feat = nc.declare_dram_parameter("features", [BL * C, HW], BF16, isOutput=False)
    mo = nc.declare_dram_parameter("main_out", [BL, C], mybir.dt.float32, isOutput=False)
    p_in = nc.declare_dram_parameter("p", [BL, HW], BF16, isOutput=False)
    out_ext = nc.declare_dram_parameter("out", [1, 1], mybir.dt.float32, isOutput=True)

    from contextlib import ExitStack

    with ExitStack() as ctx:
        e = ctx.enter_context
        m0r = e(nc.sbuf_tensor([BL * NBLK, BW], mybir.dt.float32))
        ka1 = e(nc.sbuf_tensor([BL * NBLK, BW], mybir.dt.float32))
        ka2 = e(nc.sbuf_tensor([BL * NBLK, BW], mybir.dt.float32))
        ka3 = e(nc.sbuf_tensor([BL * NBLK, BW], mybir.dt.float32))
        ca = e(nc.sbuf_tensor([BL * NBLK, 32], mybir.dt.float32))
        wba = e(nc.sbuf_tensor([BL, 256], mybir.dt.float32))
        wbb = e(nc.sbuf_tensor([BL, 256], mybir.dt.float32))
        m0 = e(nc.sbuf_tensor([BL, C], mybir.dt.float32))
        vals = e(nc.sbuf_tensor([BL, 32], mybir.dt.float32))
        idx = e(nc.sbuf_tensor([BL, 32], mybir.dt.uint32))
        idxg = e(nc.sbuf_tensor([BL, 32], mybir.dt.uint32))
        idxc0 = e(nc.sbuf_tensor([32, 1], mybir.dt.uint32))
        idxc1 = e(nc.sbuf_tensor([32, 1], mybir.dt.uint32))
        idxc2 = e(nc.sbuf_tensor([20, 1], mybir.dt.uint32))
        rowb = e(nc.sbuf_tensor([BL, 1], mybir.dt.uint32))
        gat = e(nc.sbuf_tensor([NP, HW], BF16))
        dump = e(nc.sbuf_tensor([NP, HW - SPLIT], BF16))
        colsum = e(nc.sbuf_tensor([NP, 2], mybir.dt.float32))
        ones = e(nc.sbuf_tensor([NP, 1], mybir.dt.float32))
        res = e(nc.sbuf_tensor([1, 1], mybir.dt.float32))
        warm = e(nc.sbuf_tensor([1, 1], mybir.dt.float32))
        acc = e(nc.psum_tensor([1, 2], mybir.dt.float32))
        s_mo = e(nc.semaphore())
        s_m0r = e(nc.semaphore())
        s_ck = e(nc.semaphore())
        s_p = e(nc.semaphore())
        s_out = e(nc.semaphore())
        s_gat = e(nc.semaphore())
        s_mm = e(nc.semaphore())
        s_act = e(nc.semaphore())
        s_red = e(nc.semaphore())
        s_rb = e(nc.semaphore())
        s_fl0 = e(nc.semaphore())
        s_fl1 = e(nc.semaphore())
        s_fl2 = e(nc.semaphore())
        s_dve = e(nc.semaphore())
        block = e(nc.Block())

        marks = {}
        idxcs = [idxc0, idxc1, idxc2]
        s_fls = [s_fl0, s_fl1, s_fl2]
        # chunk k (0-based) destination: rounds read A,B,A,B
        chunk_dst = [wba, wbb, wba, wbb]

        @block.sync
        def _(sync):
            sync.dma_start(m0r[:], mo[:]).then_inc(s_m0r, 16)
            sync.dma_start(m0[:], mo[:]).then_inc(s_mo, 16)
            for k in range(4):
                sync.wait_ge(s_dve, MARK_CH[k])
                sync.dma_start(
                    chunk_dst[k][:, 64 * k : 64 * k + 64], ca[:, 8 * k : 8 * k + 8]
                ).then_inc(s_ck, 16)
            for w in FL_ORDER:
                (c0, c1), nmaps, part = FWINS[w]
                sync.wait_ge(s_dve, MARK_FL[w])
                with nc.allow_non_contiguous_dma(reason="index flatten"):
                    sync.dma_start(idxcs[w][:], idxg[:, c0:c1]).then_inc(s_fls[w], 16)
            sync.wait_ge(s_red, 1)
            sync.dma_start(out_ext[:], res[:]).then_inc(s_out, 16)

        @block.vector
        def _(vector):
            n = 0

            def step(emit):
                nonlocal n
                if guard and n:
                    vector.wait_ge(s_dve, n)
                inst = emit()
                inst.then_inc(s_dve, 1)
                n += 1
                return inst

            def tta(c0, c1):
                step(
                    lambda: vector.tensor_tensor(
                        out=idxg[:, c0:c1],
                        in0=idx[:, c0:c1],
                        in1=rowb[:].to_broadcast([BL, c1 - c0]),
                        op=mybir.AluOpType.add,
                    )
                )

            def mi(lo, hi, d):
                step(
                    lambda: vector.max_index(
                        idx[0:BL, d : d + 8], vals[:, lo:hi], m0[:]
                    )
                )

            step(lambda: vector.memset(ones[:], 1.0 / DENOM))  # 1
            vector.wait_ge(s_m0r, 16)
            vector.wait_ge(s_rb, 1)
            # phase A: per-block top-32
            step(lambda: vector.max(ca[:, 0:8], m0r[:]))  # 2
            step(lambda: vector.match_replace(ka1[:], ca[:, 0:8], m0r[:], NEG))  # 3
            step(lambda: vector.max(ca[:, 8:16], ka1[:]))  # 4
            step(lambda: vector.match_replace(ka2[:], ca[:, 8:16], ka1[:], NEG))  # 5
            step(lambda: vector.max(ca[:, 16:24], ka2[:]))  # 6
            step(lambda: vector.match_replace(ka3[:], ca[:, 16:24], ka2[:], NEG))  # 7
            step(lambda: vector.max(ca[:, 24:32], ka3[:]))  # 8
            # phase B over growing per-row prefix, knock ping-pongs A->B->A->B
            vector.wait_ge(s_ck, 16)
            step(lambda: vector.max(vals[:, 0:8], wba[:, 0:64]))  # 9
            step(
                lambda: vector.match_replace(
                    wbb[:, 0:64], vals[:, 0:8], wba[:, 0:64], NEG
                )
            )  # 10
            vector.wait_ge(s_ck, 32)
            step(lambda: vector.max(vals[:, 8:16], wbb[:, 0:128]))  # 11
            vector.wait_ge(s_mo, 16)
            mi(3, 11, 0)  # 12
            tta(0, 8)  # 13
            step(
                lambda: vector.match_replace(
                    wba[:, 0:128], vals[:, 8:16], wbb[:, 0:128], NEG
                )
            )  # 14
            vector.wait_ge(s_ck, 48)
            step(lambda: vector.max(vals[:, 16:24], wba[:, 0:192]))  # 15
            mi(16, 24, 13)  # 16
            tta(16, 21)  # 17
            mi(11, 19, 8)  # 18
            tta(8, 16)  # 19
            step(
                lambda: vector.match_replace(
                    wbb[:, 0:192], vals[:, 16:24], wba[:, 0:192], NEG
                )
            )  # 20
            vector.wait_ge(s_ck, 64)
            step(lambda: vector.max(vals[:, 24:32], wbb[:, 0:256]))  # 21
            mi(17, 25, 14)  # 22
            tta(21, 22)  # 23
            assert n == MARK_G3, n
            vector.wait_ge(s_gat, 64)
            vector.wait_ge(s_p, 16)
            step(
                lambda: vector.reduce_sum(
                    colsum[:, 0:1], gat[:, 0:SPLIT], axis=mybir.AxisListType.X
                )
            )
            marks["red"] = n
            vector.wait_ge(s_mm, 1)
            if guard:
                vector.wait_ge(s_dve, n)
            vector.reduce_sum(res[:], acc[:], axis=mybir.AxisListType.X).then_inc(
                s_red, 1
            )

        @block.scalar
        def _(scalar):
            scalar.wait_ge(s_dve, 1)
            scalar.activation(
                warm[:], ones[0:1, :], mybir.ActivationFunctionType.Copy
            ).then_inc(s_act, 1)
            scalar.wait_ge(s_gat, 64)
            scalar.wait_ge(s_p, 16)
            scalar.activation(
                dump[:],
                gat[:, SPLIT:HW],
                mybir.ActivationFunctionType.Copy,
                accum_out=colsum[:, 1:2],
            ).then_inc(s_act, 1)

        @block.gpsimd
        def _(gpsimd):
            gpsimd.dma_start(gat[NGAT:NP, :], p_in[:]).then_inc(s_p, 16)
            gpsimd.iota(
                rowb[:], pattern=[[0, 1]], base=0, channel_multiplier=C
            ).then_inc(s_rb, 1)
            for w in FL_ORDER:
                (c0, c1), nmaps, part = FWINS[w]
                gpsimd.wait_ge(s_fls[w], 16)
                gpsimd.indirect_dma_start(
                    out=gat[part : part + nmaps, :],
                    out_offset=None,
                    in_=feat[:],
                    in_offset=bass.IndirectOffsetOnAxis(ap=idxcs[w][:], axis=0),
                ).then_inc(s_gat, 16)
            # final window: rank 24 of each row -- [4,1] offsets, no flatten
            gpsimd.wait_ge(s_dve, MARK_G3)
            gpsimd.indirect_dma_start(
                out=gat[84:88, :],
                out_offset=None,
                in_=feat[:],
                in_offset=bass.IndirectOffsetOnAxis(ap=idxg[:, 21:22], axis=0),
            ).then_inc(s_gat, 16)

        @block.tensor
        def _(tensor):
            tensor.wait_ge(s_dve, marks["red"])
            tensor.wait_ge(s_act, 2)
            tensor.matmul(acc[:], ones[:], colsum[:]).then_inc(s_mm, 1)

    return nc


def shard_inputs(p, main_out, features):
    p16 = p.astype(ml_dtypes.bfloat16)
    f16 = features.astype(ml_dtypes.bfloat16)
    in_maps = []
    for i in range(NCORES):
        sl = slice(i * BL, (i + 1) * BL)
        in_maps.append(
            {
                "features": f16[sl].reshape(BL * C, HW),
                "main_out": main_out[sl],
                "p": p16[sl].reshape(BL, HW),
            }
        )
    return in_maps


def kernel(p, main_out, features, return_res=False, guard=True):
    p = np.ascontiguousarray(np.asarray(p, dtype=np.float32))
    main_out = np.ascontiguousarray(np.asarray(main_out, dtype=np.float32))
    features = np.ascontiguousarray(np.asarray(features, dtype=np.float32))

    nc = build_nc(guard=guard)
    in_maps = shard_inputs(p, main_out, features)
    res = run_bass_kernel_spmd(nc, in_maps, core_ids=list(range(NCORES)))
    total = np.float32(0.0)
    for r in res.results:
        total += r["out"][0, 0]
    out = np.asarray(total, dtype=np.float32)
    if return_res:
        return out, res
    return out
